# revision 1
# baseline (speedup 1.0000x reference)
"""Trainium2 Bass kernel for nn_Attention (dense transformer attention block).

Full inputs -> full output. Internally: 8 NeuronCores, 2 data-parallel groups
(batch) x 4-way tensor-parallel (heads). Each core computes 8 heads for one
batch element; wo partial sums are combined with a grouped ReduceScatter over
8 sequence slices (short collective tail).

Compute in bf16 on the TensorEngine (fp32 matmul is 4x slower), fp32 PSUM
accumulation. All operand layouts are pre-rearranged host-side so every
device DMA is a contiguous per-partition block:
  - projections:  qT/kT = (w-tile).T @ xhatT-tile   -> [feature, seq] layout
  - scores:       scoresT[t, s] = kT-tile.T @ qT    (softmax along partitions)
  - Z:            ones[128,128].T @ probs           -> Z broadcast to all rows
  - PV:           oT[dh, s] = v-tile.T @ probsT
  - wo:           y[s, d] = oT-tile.T @ wo-slab
Causal structure is exploited: score tiles that are fully masked are skipped
(scores/exp/Z/PV), and only the 4 diagonal-band tiles per query chunk get a
post-exp 0/1 multiply (from 4 precomputed [128,512] masks). RoPE pairs are
deinterleaved host-side (even dims first); the 1/sqrt(128) score scale is
folded into the q-side cos/sin tables.
"""

import sys

import numpy as np

for _p in ("/opt/trn_rl_repo",):
    if _p not in sys.path:
        sys.path.insert(0, _p)

import ml_dtypes

BF16 = ml_dtypes.bfloat16

D = 4096      # model dim
S = 1024      # decoder sequence length
E = 512       # encoder length
T = E + S     # total key length
H = 8         # heads per core (32 total / 4-way TP)
DH = 128      # head dim
O = H * DH    # per-core projection width = 1024
NDT = D // 128
NEG = -1e9
REPLICA_GROUPS = [[0, 1, 2, 3], [4, 5, 6, 7]]

_CACHE = {}
LAST_EXEC_NS = None


def _build(no_collective=False):
    import concourse.mybir as mybir
    import concourse.tile as tile
    from concourse import bacc

    bf16 = mybir.dt.bfloat16

    nc = bacc.Bacc(
        "TRN2",
        target_bir_lowering=False,
        debug=False,
        num_devices=8,
    )

    P = {}
    for name, shape in [
        ("x_r", [128, NDT * T]),        # xhatT slabs: cols dt*T + t
        ("wq_r", [128, NDT * O]),       # Q pass slabs: cols p*8192 + n*256 + c
        ("wk_r", [128, NDT * O]),       # K pass slabs: same geometry
        ("wv_r", [128, NDT * O]),       # V slabs: cols oc*16384 + n*512 + c
        ("wo_r", [128, H * D]),         # wo slabs: cols h*D + d
        ("csq_cos", [128, S]),
        ("csq_sin", [128, S]),
        ("csk_cos", [128, S]),
        ("csk_sin", [128, S]),
        ("dmask", [128, 4 * 512]),      # 4 diagonal-band masks
        ("ones", [128, 128]),
    ]:
        P[name] = nc.declare_dram_parameter(name, shape, bf16, isOutput=False)
    out = nc.declare_dram_parameter("out", [256, D], bf16, isOutput=True)

    with tile.TileContext(nc) as tc:
        _emit(nc, tc, P, out, no_collective=no_collective)
    nc.compile()
    return nc


def _emit(nc, tc, P, out, no_collective=False):
    import concourse.mybir as mybir
    from concourse.bass import ds

    bf16 = mybir.dt.bfloat16
    fp32 = mybir.dt.float32
    AF = mybir.ActivationFunctionType

    with tc.tile_pool(name="res", bufs=1) as res, \
         tc.tile_pool(name="dram", bufs=1, space="DRAM") as dram:
        qT = res.tile([128, H * S], bf16, tag="qT")     # cols h*S + s
        kT = res.tile([128, H * T], bf16, tag="kT")     # cols h*T + t
        vsb = res.tile([128, 12 * O], bf16, tag="vsb")  # cols tt*O + o
        onesb = res.tile([128, 128], bf16, tag="onesb")
        dmsk = res.tile([128, 4 * 512], bf16, tag="dmsk")  # 0/1 keep masks

        # ReduceScatter slices: s-tile groups of [2,2,2,1,1] (big first for
        # stream bandwidth, small last for a short tail)
        RS_ST = [(0, 2), (2, 2), (4, 2), (6, 1), (7, 1)]
        y_dram = dram.tile([S, D], bf16, tag="y_dram")
        rs_out = [
            dram.tile([n * 32, D], bf16, tag=f"rs{i}", name=f"rs_out{i}")
            for i, (_, n) in enumerate(RS_ST)
        ]

        nc.gpsimd.dma_start(out=dmsk[:, :], in_=P["dmask"][:, :])
        nc.gpsimd.dma_start(out=onesb[:, :], in_=P["ones"][:, :])

        # ---------------- phase 1: projections + rope ----------------
        with tc.tile_pool(name="xpool", bufs=1) as xpool, \
             tc.tile_pool(name="tabpool", bufs=1) as tabpool, \
             tc.tile_pool(name="wpool", bufs=3) as wpool, \
             tc.tile_pool(name="rtmp", bufs=2) as rtmp, \
             tc.tile_pool(name="ps1", bufs=8, space="PSUM") as ps1:
            # first Q weight half-slab ahead of everything on the sync queue
            def wslab(src, off, n, name):
                wr = wpool.tile([128, n], bf16, tag="wr", name=name)
                nc.sync.dma_start(out=wr[:, :], in_=P[src][:, ds(off, n)])
                return wr

            wr_q00 = wslab("wq_r", 0, 16 * 256, "wr_q0_0")
            xh = []
            for dt in range(NDT):
                xt = xpool.tile([128, T], bf16, tag=f"xh{dt}", name=f"xh{dt}")
                (nc.scalar if dt % 2 == 0 else nc.gpsimd).dma_start(
                    out=xt[:, :], in_=P["x_r"][:, ds(dt * T, T)]
                )
                xh.append(xt)
            csqc = tabpool.tile([128, S], bf16, tag="csqc")
            csqs = tabpool.tile([128, S], bf16, tag="csqs")
            cskc = tabpool.tile([128, S], bf16, tag="cskc")
            csks = tabpool.tile([128, S], bf16, tag="csks")
            nc.scalar.dma_start(out=csqc[:, :], in_=P["csq_cos"][:, :])
            nc.scalar.dma_start(out=csqs[:, :], in_=P["csq_sin"][:, :])
            nc.gpsimd.dma_start(out=cskc[:, :], in_=P["csk_cos"][:, :])
            nc.gpsimd.dma_start(out=csks[:, :], in_=P["csk_sin"][:, :])

            # RoPE: tables are full-height with the 64-row block duplicated
            # (cos) or sign-split (-sin; +sin), so every TensorTensor is
            # partition-aligned. The half-swap goes through an SBUF-SBUF DMA.
            def rope(buf, base, cos, sin, tag):
                swp = rtmp.tile([128, S], bf16, tag="swp", name=f"swp_{tag}")
                nc.gpsimd.dma_start(
                    out=swp[ds(0, 64), :], in_=buf[ds(64, 64), ds(base, S)]
                )
                nc.gpsimd.dma_start(
                    out=swp[ds(64, 64), :], in_=buf[ds(0, 64), ds(base, S)]
                )
                nc.vector.tensor_mul(swp[:, :], swp[:, :], sin[:, :])
                nc.vector.tensor_mul(
                    buf[:, ds(base, S)], buf[:, ds(base, S)], cos[:, :]
                )
                nc.vector.tensor_add(
                    buf[:, ds(base, S)], buf[:, ds(base, S)], swp[:, :]
                )

            # Q: 4 passes x (2 o_tiles x 2 s_chunks); K: 4 passes x
            # (2 o_tiles x 3 t_chunks). Weight slabs stream in 16-dt halves.
            for src, nch, xoff, obuf, ostride in (
                ("wq_r", 2, E, qT, S),
                ("wk_r", 3, 0, kT, T),
            ):
                for p in range(4):
                    ps = [
                        [ps1.tile([128, 512], fp32, tag="ps1",
                                  name=f"ps_{src}_{p}_{oi}_{cc}")
                         for cc in range(nch)]
                        for oi in range(2)
                    ]
                    for half in range(2):
                        if src == "wq_r" and p == 0 and half == 0:
                            wr = wr_q00
                        else:
                            wr = wslab(
                                src, (p * 2 + half) * 16 * 256, 16 * 256,
                                f"wr_{src}_{p}_{half}",
                            )
                        for dtl in range(16):
                            dt = half * 16 + dtl
                            for oi in range(2):
                                for cc in range(nch):
                                    nc.tensor.matmul(
                                        ps[oi][cc][:, :],
                                        wr[:, ds(dtl * 256 + oi * 128, 128)],
                                        xh[dt][:, ds(xoff + cc * 512, 512)],
                                        start=(dt == 0),
                                        stop=(dt == NDT - 1),
                                    )
                    for oi in range(2):
                        h = 2 * p + oi
                        for cc in range(nch):
                            nc.scalar.copy(
                                obuf[:, ds(h * ostride + cc * 512, 512)],
                                ps[oi][cc][:, :],
                            )
                        if src == "wq_r":
                            rope(qT, h * S, csqc, csqs, f"q{h}")
                        else:
                            rope(kT, h * T + E, cskc, csks, f"k{h}")

            # V (x-stationary): 2 o_chunks x 2 t_groups of 6 tiles; weight
            # slabs re-streamed per t_group in two 16-dt halves
            for oc in range(2):
                for tg in range(2):
                    tb = tg * 6
                    psv = [ps1.tile([128, 512], fp32, tag="ps1",
                                    name=f"psv_{oc}_{tg}_{ti}")
                           for ti in range(6)]
                    for qr in range(4):
                        wr = wpool.tile(
                            [128, 8 * 512], bf16, tag="wr",
                            name=f"wr_v{oc}_{tg}_{qr}",
                        )
                        nc.sync.dma_start(
                            out=wr[:, :],
                            in_=P["wv_r"][
                                :, ds(oc * NDT * 512 + qr * 8 * 512, 8 * 512)
                            ],
                        )
                        for dtl in range(8):
                            dt = qr * 8 + dtl
                            for ti in range(6):
                                nc.tensor.matmul(
                                    psv[ti][:, :],
                                    xh[dt][:, ds((tb + ti) * 128, 128)],
                                    wr[:, ds(dtl * 512, 512)],
                                    start=(dt == 0),
                                    stop=(dt == NDT - 1),
                                )
                    for ti in range(6):
                        nc.scalar.copy(
                            vsb[:, ds((tb + ti) * O + oc * 512, 512)],
                            psv[ti][:, :],
                        )

        # -------- phase 2: attention (softmax along partitions) --------
        # Per (sc, h): tile list = 4 encoder tiles + decoder tiles that are
        # not fully masked (sc0: 4, sc1: 8). Scores into paired psum banks,
        # exp over the pair, Z via ones-stationary matmul (broadcast to all
        # partitions), PV accumulation, then one reciprocal + one mul.
        with tc.tile_pool(name="opool", bufs=1) as opool, \
             tc.tile_pool(name="wopool", bufs=1) as wopool:
          oT = opool.tile([128, H * S], bf16, tag="oT")  # cols h*S + s
          wosb = wopool.tile([128, H * D], bf16, tag="wo")
          for q4 in range(4):
              (nc.scalar if q4 % 2 == 0 else nc.gpsimd).dma_start(
                  out=wosb[:, ds(q4 * 2 * D, 2 * D)],
                  in_=P["wo_r"][:, ds(q4 * 2 * D, 2 * D)],
              )
          with tc.tile_pool(name="ppool", bufs=3) as ppool, \
               tc.tile_pool(name="zpool", bufs=2) as zpool, \
               tc.tile_pool(name="psS", bufs=3, space="PSUM") as psS, \
               tc.tile_pool(name="psZ", bufs=1, space="PSUM") as psZ, \
               tc.tile_pool(name="psV", bufs=1, space="PSUM") as psV:
            def tiles_for(sc):
                # (tt, diag_j): tt indexes kT/vsb t-tiles; diag_j is the
                # diagonal-mask index or None. Fully-masked tiles skipped.
                lst = [(tt, None) for tt in range(4)]  # encoder
                if sc == 0:
                    lst += [(4 + j, j) for j in range(4)]
                else:
                    lst += [(tt, None) for tt in range(4, 8)]
                    lst += [(8 + j, j) for j in range(4)]
                return lst

            def emit_A(sc, h, pbuf):
                tl = tiles_for(sc)
                for k0 in range(0, len(tl), 2):
                    pr = psS.tile([128, 1024], fp32, tag="psS",
                                  name=f"sc{sc}h{h}p{k0}")
                    for half in range(2):
                        tt, dj = tl[k0 + half]
                        nc.tensor.matmul(
                            pr[:, ds(half * 512, 512)],
                            kT[:, ds(h * T + tt * 128, 128)],
                            qT[:, ds(h * S + sc * 512, 512)],
                            start=True,
                            stop=True,
                        )
                    nc.scalar.activation(
                        pbuf[:, ds(k0 * 512, 1024)], pr[:, :], AF.Exp
                    )
                    # causal zeroing of the diagonal-band tiles, post-exp
                    for half in range(2):
                        tt, dj = tl[k0 + half]
                        if dj is not None:
                            nc.vector.tensor_mul(
                                pbuf[:, ds((k0 + half) * 512, 512)],
                                pbuf[:, ds((k0 + half) * 512, 512)],
                                dmsk[:, ds(dj * 512, 512)],
                            )

            def emit_B(sc, h, pbuf):
                tl = tiles_for(sc)
                n = len(tl)
                zp = psZ.tile([128, 512], fp32, tag="psZ", name=f"z{sc}{h}")
                for k, (tt, _) in enumerate(tl):
                    nc.tensor.matmul(
                        zp[:, :],
                        onesb[:, :],
                        pbuf[:, ds(k * 512, 512)],
                        start=(k == 0),
                        stop=(k == n - 1),
                    )
                zr = zpool.tile([128, 512], fp32, tag="zr", name=f"zr{sc}{h}")
                nc.vector.reciprocal_approx_fast(zr[:, :], zp[:, :])
                pv = psV.tile([128, 512], fp32, tag="psV", name=f"pv{sc}{h}")
                for k, (tt, _) in enumerate(tl):
                    nc.tensor.matmul(
                        pv[:, :],
                        vsb[:, ds(tt * O + h * 128, 128)],
                        pbuf[:, ds(k * 512, 512)],
                        start=(k == 0),
                        stop=(k == n - 1),
                    )
                nc.vector.tensor_mul(
                    oT[:, ds(h * S + sc * 512, 512)], pv[:, :], zr[:, :]
                )

            # wo chains share the psZ/psV banks between attention uses.
            # st0-3 (sc0 rows) are interleaved into the sc1 A-slots so the
            # ReduceScatter stream starts ~100us before attention finishes.
            slice_of_st = {}
            for i, (sst, n) in enumerate(RS_ST):
                for st in range(sst, sst + n):
                    slice_of_st[st] = (i, sst, n)
            out_off = [0]
            for _, n in RS_ST:
                out_off.append(out_off[-1] + n * 32)

            with tc.tile_pool(name="ypool", bufs=2) as ypool:
                yrows = {}

                def emit_wo_chain(idx):
                    st, dc = divmod(idx, 8)
                    if dc == 0:
                        yrows[st] = ypool.tile(
                            [128, D], bf16, tag="y", name=f"yrow{st}"
                        )
                    pool = psZ if idx % 2 == 0 else psV
                    py = pool.tile([128, 512], fp32,
                                   tag="psZ" if idx % 2 == 0 else "psV",
                                   name=f"py{st}_{dc}")
                    for h in range(H):
                        nc.tensor.matmul(
                            py[:, :],
                            oT[:, ds(h * S + st * 128, 128)],
                            wosb[:, ds(h * D + dc * 512, 512)],
                            start=(h == 0),
                            stop=(h == H - 1),
                        )
                    nc.scalar.copy(yrows[st][:, ds(dc * 512, 512)], py[:, :])
                    if dc == 7:
                        nc.sync.dma_start(
                            out=y_dram[ds(st * 128, 128), :],
                            in_=yrows[st][:, :],
                        )
                        i, sst, n = slice_of_st[st]
                        if st == sst + n - 1:
                            if not no_collective:
                                nc.gpsimd.collective_compute(
                                    "ReduceScatter",
                                    mybir.AluOpType.add,
                                    replica_groups=REPLICA_GROUPS,
                                    ins=[y_dram[ds(sst * 128, n * 128), :].opt()],
                                    outs=[rs_out[i][:, :].opt()],
                                )
                            # gpsimd queue: serializes only against the
                            # collectives this DMA already depends on —
                            # keeps the CC wait out of the sync-queue FIFO
                            nc.gpsimd.dma_start(
                                out=out[ds(out_off[i], n * 32), :],
                                in_=rs_out[i][:, :],
                            )

                # software pipeline: 2-head lookahead; after the 8th emitted
                # B (all sc0 oT ready) start slipping wo chains in.
                WO_BUDGET = [5, 5, 5, 5, 4, 4, 4]  # per B#7..B#13
                pend = []
                b_count = 0
                wo_idx = 0
                for sc in range(2):
                    for h in range(H):
                        pbuf = ppool.tile(
                            [128, 12 * 512], bf16, tag="p", name=f"pb{sc}{h}"
                        )
                        emit_A(sc, h, pbuf)
                        pend.append((sc, h, pbuf))
                        if len(pend) == 3:
                            s0, h0, pb0 = pend.pop(0)
                            emit_B(s0, h0, pb0)
                            if b_count >= 7:
                                for _ in range(WO_BUDGET[b_count - 7]):
                                    emit_wo_chain(wo_idx)
                                    wo_idx += 1
                            b_count += 1
                for s0, h0, pb0 in pend:
                    emit_B(s0, h0, pb0)
                    b_count += 1
                # ---------------- phase 3: wo st4-7 + RS tail ----------------
                while wo_idx < 64:
                    emit_wo_chain(wo_idx)
                    wo_idx += 1


def _prep_in_maps(x, freqs_cos, freqs_sin, mask, encoder_output, wq, wk, wv, wo):
    x = np.asarray(x, np.float32)
    encoder_output = np.asarray(encoder_output, np.float32)
    freqs_cos = np.asarray(freqs_cos, np.float32)
    freqs_sin = np.asarray(freqs_sin, np.float32)
    wq = np.asarray(wq, np.float32)
    wk = np.asarray(wk, np.float32)
    wv = np.asarray(wv, np.float32)
    wo = np.asarray(wo, np.float32)

    def perm(w):  # deinterleave rope pairs per head: even dims first
        w4 = w.reshape(H, 64, 2, D)
        return np.ascontiguousarray(w4.transpose(0, 2, 1, 3)).reshape(O, D)

    def slab256(wT):  # [D, O] -> [128, 4*32*256]: pass p, dt n, col c
        w4 = wT.reshape(NDT, 128, 4, 256)            # [n, part, p, c]
        return np.ascontiguousarray(
            w4.transpose(1, 2, 0, 3)
        ).reshape(128, NDT * O)

    def slab512(wT):  # [D, O] -> [128, 2*32*512]: oc, dt n, col c
        w4 = wT.reshape(NDT, 128, 2, 512)
        return np.ascontiguousarray(
            w4.transpose(1, 2, 0, 3)
        ).reshape(128, NDT * O)

    alpha = 1.0 / np.sqrt(DH)
    cosT = freqs_cos.T  # [64, S]
    sinT = freqs_sin.T
    csq_cos = (np.concatenate([cosT, cosT], 0) * alpha).astype(BF16)
    csq_sin = (np.concatenate([-sinT, sinT], 0) * alpha).astype(BF16)
    csk_cos = np.concatenate([cosT, cosT], 0).astype(BF16)
    csk_sin = np.concatenate([-sinT, sinT], 0).astype(BF16)

    # 4 diagonal-band keep-masks (0/1, applied post-exp):
    # dmask[t, j*512+s] = 0 if s < t + j*128 else 1
    t_i = np.arange(128)[:, None]
    s_i = np.arange(512)[None, :]
    dmask = np.concatenate(
        [np.where(s_i < t_i + j * 128, 0.0, 1.0) for j in range(4)], axis=1
    ).astype(BF16)
    ones = np.ones((128, 128), BF16)

    in_maps = []
    for c in range(8):
        g, r = divmod(c, 4)
        sl = slice(r * O, (r + 1) * O)
        xhat = np.concatenate([encoder_output[g], x[g]], axis=0)  # [T, D]
        xhatT = xhat.T.astype(BF16)                               # [D, T]
        x_r = np.ascontiguousarray(
            xhatT.reshape(NDT, 128, T).transpose(1, 0, 2)
        ).reshape(128, NDT * T)
        wqT = perm(wq[sl]).T.astype(BF16)   # [D, O]
        wkT = perm(wk[sl]).T.astype(BF16)
        wvT = wv[sl].T.astype(BF16)
        woT = wo[:, sl].T.astype(BF16)      # [O, D]
        wo_r = np.ascontiguousarray(
            woT.reshape(H, 128, D).transpose(1, 0, 2)
        ).reshape(128, H * D)
        in_maps.append(
            {
                "x_r": x_r,
                "wq_r": slab256(wqT),
                "wk_r": slab256(wkT),
                "wv_r": slab512(wvT),
                "wo_r": wo_r,
                "csq_cos": csq_cos,
                "csq_sin": csq_sin,
                "csk_cos": csk_cos,
                "csk_sin": csk_sin,
                "dmask": dmask,
                "ones": ones,
            }
        )
    return in_maps


RS_SLICES = [(0, 2), (2, 2), (4, 2), (6, 1), (7, 1)]  # (s-tile start, count)


def _gather(outs):
    full = np.zeros((2, S, D), np.float32)
    for c in range(8):
        g, r = divmod(c, 4)
        o = np.asarray(outs[c]).astype(np.float32)
        off = 0
        for st0, n in RS_SLICES:
            rows = n * 32  # per-core rows for this slice
            y0 = st0 * 128 + r * rows
            full[g, y0: y0 + rows] = o[off: off + rows]
            off += rows
    return full


def kernel(x, start_pos, freqs_cos, freqs_sin, mask, encoder_output, wq, wk, wv, wo):
    global LAST_EXEC_NS
    from concourse.bass_utils import run_bass_kernel_spmd

    if "nc" not in _CACHE:
        _CACHE["nc"] = _build()
    nc = _CACHE["nc"]

    in_maps = _prep_in_maps(
        x, freqs_cos, freqs_sin, mask, encoder_output, wq, wk, wv, wo
    )
    res = run_bass_kernel_spmd(nc, in_maps, core_ids=list(range(8)))
    LAST_EXEC_NS = res.exec_time_ns
    return _gather([res.results[c]["out"] for c in range(8)])



# revision 8
# speedup vs baseline: 1.0237x; 1.0237x over previous
"""Trainium2 Bass kernel for nn_Attention (dense transformer attention block).

Full inputs -> full output. Internally: 8 NeuronCores, 2 data-parallel groups
(batch) x 4-way tensor-parallel (heads). Each core computes 8 heads for one
batch element. The wo projection is redistributed with a single 8-rank
AllToAll per 512-token slice: each core ships its heads' attention output
(oT, feature-major) for token-quarter q to ranks q and q+4, and afterwards
holds the FULL 4096-feature oT for one 128-token quarter of each batch --
it then computes y for those rows over one 2048-wide d-half (group 0 takes
d 0:2048, group 1 d 2048:4096). No ReduceScatter; the A2A moves 2 MB/rank
at ~24 us (mesh) and the sc1 A2A hides under the sc0 wo pass.

Compute in bf16 on the TensorEngine (fp32 matmul is 4x slower), fp32 PSUM
accumulation. All operand layouts are pre-rearranged host-side so every
device DMA is a contiguous per-partition block:
  - projections:  qT/kT = (w-tile).T @ xhatT-tile   -> [feature, seq] layout
  - scores:       scoresT[t, s] = kT-tile.T @ qT    (softmax along partitions)
  - Z:            ones[128,128].T @ probs           -> Z broadcast to all rows
  - PV:           oT[dh, s] = v-tile.T @ probsT
  - wo:           y[t, d] = oT-recv-tile.T @ woT-slab (K=4096 in one psum)
Causal structure is exploited: score tiles that are fully masked are skipped
(scores/exp/Z/PV), and only the 4 diagonal-band tiles per query chunk get a
post-exp 0/1 multiply (from 4 precomputed [128,512] masks). RoPE pairs are
deinterleaved host-side (even dims first); the 1/sqrt(128) score scale is
folded into the q-side cos/sin tables.
"""

import sys

import numpy as np

for _p in ("/opt/trn_rl_repo",):
    if _p not in sys.path:
        sys.path.insert(0, _p)

import ml_dtypes

BF16 = ml_dtypes.bfloat16

D = 4096      # model dim
S = 1024      # decoder sequence length
E = 512       # encoder length
T = E + S     # total key length
H = 8         # heads per core (32 total / 4-way TP)
DH = 128      # head dim
O = H * DH    # per-core projection width = 1024
NDT = D // 128
NEG = -1e9
A2A_GROUP = [[0, 1, 2, 3, 4, 5, 6, 7]]
DHALF = 2048  # per-core output d-half width

_CACHE = {}
LAST_EXEC_NS = None


def _build(no_collective=False):
    import concourse.mybir as mybir
    import concourse.tile as tile
    from concourse import bacc

    bf16 = mybir.dt.bfloat16

    nc = bacc.Bacc(
        "TRN2",
        target_bir_lowering=False,
        debug=False,
        num_devices=8,
    )

    P = {}
    for name, shape in [
        ("x_r", [128, NDT * T]),        # xhatT slabs: cols dt*T + t
        ("wq_r", [128, NDT * O]),       # Q pass slabs: cols p*8192 + n*256 + c
        ("wk_r", [128, NDT * O]),       # K pass slabs: same geometry
        ("wv_r", [128, NDT * O]),       # V slabs: cols oc*16384 + n*512 + c
        ("wo_r", [128, 2 * 32 * 1024]), # woT slabs: cols p*32768 + kt*1024 + d
        ("csq_cos", [128, S]),
        ("csq_sin", [128, S]),
        ("csk_cos", [128, S]),
        ("csk_sin", [128, S]),
        ("dmask", [128, 4 * 512]),      # 4 diagonal-band masks
        ("ones", [128, 128]),
    ]:
        P[name] = nc.declare_dram_parameter(name, shape, bf16, isOutput=False)
    # rows: sc*256 + b*128 + t ; cols: d within this core's d-half
    out = nc.declare_dram_parameter("out", [512, DHALF], bf16, isOutput=True)

    with tile.TileContext(nc) as tc:
        _emit(nc, tc, P, out, no_collective=no_collective)
    nc.compile()
    return nc


def _emit(nc, tc, P, out, no_collective=False):
    import concourse.mybir as mybir
    from concourse.bass import ds

    bf16 = mybir.dt.bfloat16
    fp32 = mybir.dt.float32
    AF = mybir.ActivationFunctionType

    with tc.tile_pool(name="res", bufs=1) as res, \
         tc.tile_pool(name="dram", bufs=1, space="DRAM") as dram:
        onesb = res.tile([128, 128], bf16, tag="onesb")
        dmsk = res.tile([128, 4 * 512], bf16, tag="dmsk")  # 0/1 keep masks

        # A2A staging: rows j*128 + dh (j = dest rank), cols h*128 + t
        a2a_in = [
            dram.tile([1024, 1024], bf16, tag=f"ai{sc}", name=f"a2a_in{sc}")
            for sc in range(2)
        ]
        a2a_out = [
            dram.tile([1024, 1024], bf16, tag=f"ao{sc}", name=f"a2a_out{sc}")
            for sc in range(2)
        ]

        nc.gpsimd.dma_start(out=dmsk[:, :], in_=P["dmask"][:, :])
        nc.gpsimd.dma_start(out=onesb[:, :], in_=P["ones"][:, :])

        with tc.tile_pool(name="qkv", bufs=1) as qkv:
          qT = qkv.tile([128, H * S], bf16, tag="qT")     # cols h*S + s
          kT = qkv.tile([128, H * T], bf16, tag="kT")     # cols h*T + t
          vsb = qkv.tile([128, 12 * O], bf16, tag="vsb")  # cols tt*O + o

          # ---------------- phase 1: projections + rope ----------------
          with tc.tile_pool(name="xpool", bufs=1) as xpool, \
               tc.tile_pool(name="tabpool", bufs=1) as tabpool, \
               tc.tile_pool(name="wpool", bufs=3) as wpool, \
               tc.tile_pool(name="rtmp", bufs=2) as rtmp, \
               tc.tile_pool(name="ps1", bufs=8, space="PSUM") as ps1:
            # first Q weight half-slab ahead of everything on the sync queue
            def wslab(src, off, n, name):
                wr = wpool.tile([128, n], bf16, tag="wr", name=name)
                nc.sync.dma_start(out=wr[:, :], in_=P[src][:, ds(off, n)])
                return wr

            wr_q00 = wslab("wq_r", 0, 16 * 256, "wr_q0_0")
            xh = []
            for dt in range(NDT):
                xt = xpool.tile([128, T], bf16, tag=f"xh{dt}", name=f"xh{dt}")
                (nc.scalar if dt % 2 == 0 else nc.gpsimd).dma_start(
                    out=xt[:, :], in_=P["x_r"][:, ds(dt * T, T)]
                )
                xh.append(xt)
            csqc = tabpool.tile([128, S], bf16, tag="csqc")
            csqs = tabpool.tile([128, S], bf16, tag="csqs")
            cskc = tabpool.tile([128, S], bf16, tag="cskc")
            csks = tabpool.tile([128, S], bf16, tag="csks")
            nc.scalar.dma_start(out=csqc[:, :], in_=P["csq_cos"][:, :])
            nc.scalar.dma_start(out=csqs[:, :], in_=P["csq_sin"][:, :])
            nc.gpsimd.dma_start(out=cskc[:, :], in_=P["csk_cos"][:, :])
            nc.gpsimd.dma_start(out=csks[:, :], in_=P["csk_sin"][:, :])

            # RoPE: tables are full-height with the 64-row block duplicated
            # (cos) or sign-split (-sin; +sin), so every TensorTensor is
            # partition-aligned. The half-swap goes through an SBUF-SBUF DMA.
            def rope(buf, base, cos, sin, tag):
                swp = rtmp.tile([128, S], bf16, tag="swp", name=f"swp_{tag}")
                nc.gpsimd.dma_start(
                    out=swp[ds(0, 64), :], in_=buf[ds(64, 64), ds(base, S)]
                )
                nc.gpsimd.dma_start(
                    out=swp[ds(64, 64), :], in_=buf[ds(0, 64), ds(base, S)]
                )
                nc.vector.tensor_mul(swp[:, :], swp[:, :], sin[:, :])
                nc.vector.tensor_mul(
                    buf[:, ds(base, S)], buf[:, ds(base, S)], cos[:, :]
                )
                nc.vector.tensor_add(
                    buf[:, ds(base, S)], buf[:, ds(base, S)], swp[:, :]
                )

            # Q: 4 passes x (2 o_tiles x 2 s_chunks); K: 4 passes x
            # (2 o_tiles x 3 t_chunks). Weight slabs stream in 16-dt halves.
            for src, nch, xoff, obuf, ostride in (
                ("wq_r", 2, E, qT, S),
                ("wk_r", 3, 0, kT, T),
            ):
                for p in range(4):
                    ps = [
                        [ps1.tile([128, 512], fp32, tag="ps1",
                                  name=f"ps_{src}_{p}_{oi}_{cc}")
                         for cc in range(nch)]
                        for oi in range(2)
                    ]
                    for half in range(2):
                        if src == "wq_r" and p == 0 and half == 0:
                            wr = wr_q00
                        else:
                            wr = wslab(
                                src, (p * 2 + half) * 16 * 256, 16 * 256,
                                f"wr_{src}_{p}_{half}",
                            )
                        for dtl in range(16):
                            dt = half * 16 + dtl
                            for oi in range(2):
                                for cc in range(nch):
                                    nc.tensor.matmul(
                                        ps[oi][cc][:, :],
                                        wr[:, ds(dtl * 256 + oi * 128, 128)],
                                        xh[dt][:, ds(xoff + cc * 512, 512)],
                                        start=(dt == 0),
                                        stop=(dt == NDT - 1),
                                    )
                    for oi in range(2):
                        h = 2 * p + oi
                        for cc in range(nch):
                            nc.scalar.copy(
                                obuf[:, ds(h * ostride + cc * 512, 512)],
                                ps[oi][cc][:, :],
                            )
                        if src == "wq_r":
                            rope(qT, h * S, csqc, csqs, f"q{h}")
                        else:
                            rope(kT, h * T + E, cskc, csks, f"k{h}")

            # V (x-stationary): 2 o_chunks x 2 t_groups of 6 tiles; weight
            # slabs re-streamed per t_group in two 16-dt halves
            for oc in range(2):
                for tg in range(2):
                    tb = tg * 6
                    psv = [ps1.tile([128, 512], fp32, tag="ps1",
                                    name=f"psv_{oc}_{tg}_{ti}")
                           for ti in range(6)]
                    for qr in range(4):
                        wr = wpool.tile(
                            [128, 8 * 512], bf16, tag="wr",
                            name=f"wr_v{oc}_{tg}_{qr}",
                        )
                        nc.sync.dma_start(
                            out=wr[:, :],
                            in_=P["wv_r"][
                                :, ds(oc * NDT * 512 + qr * 8 * 512, 8 * 512)
                            ],
                        )
                        for dtl in range(8):
                            dt = qr * 8 + dtl
                            for ti in range(6):
                                nc.tensor.matmul(
                                    psv[ti][:, :],
                                    xh[dt][:, ds((tb + ti) * 128, 128)],
                                    wr[:, ds(dtl * 512, 512)],
                                    start=(dt == 0),
                                    stop=(dt == NDT - 1),
                                )
                    for ti in range(6):
                        nc.scalar.copy(
                            vsb[:, ds((tb + ti) * O + oc * 512, 512)],
                            psv[ti][:, :],
                        )

          # wo pass-1 slabs + sc0 recv tiles prefetch during attention; this
          # pool sits in the (dead) phase-1 x region so its DMAs only wait on
          # the last projection matmul, not on attention.
          with tc.tile_pool(name="wpre", bufs=1) as wpre:
            slab1 = [
                wpre.tile([128, 1024], bf16, tag=f"sl1_{kt}", name=f"slab1_{kt}")
                for kt in range(32)
            ]
            rt0 = [
                wpre.tile([128, 1024], bf16, tag=f"rt0_{i}", name=f"rt0_{i}")
                for i in range(8)
            ]
            # stream pass-1 slabs during attention; gpsimd issues these
            # before the A2A trigger queues behind them, scalar slots in
            # between exp activations (no waits: the region's phase-1
            # tenants are already dead)
            for kt in range(32):
                (nc.gpsimd if kt % 2 == 0 else nc.scalar).dma_start(
                    out=slab1[kt][:, :],
                    in_=P["wo_r"][:, ds(kt * 1024, 1024)],
                )

            # -------- phase 2: attention (softmax along partitions) --------
            # Per (sc, h): tile list = 4 encoder tiles + decoder tiles that
            # are not fully masked (sc0: 4, sc1: 8). Scores into paired psum
            # banks, exp over the pair, Z via ones-stationary matmul, PV
            # accumulation, then one reciprocal + one mul. After each head's
            # oT is ready it is staged to the A2A input (quarters duplicated
            # to ranks q and q+4); the A2A for a slice fires after its 8th
            # head.
            with tc.tile_pool(name="opool", bufs=1) as opool, \
                 tc.tile_pool(name="ppool", bufs=3) as ppool, \
                 tc.tile_pool(name="zpool", bufs=2) as zpool, \
                 tc.tile_pool(name="psS", bufs=3, space="PSUM") as psS, \
                 tc.tile_pool(name="psZ", bufs=1, space="PSUM") as psZ, \
                 tc.tile_pool(name="psV", bufs=1, space="PSUM") as psV:
              oT = opool.tile([128, H * S], bf16, tag="oT")  # cols h*S + s

              def tiles_for(sc):
                  # (tt, diag_j): tt indexes kT/vsb t-tiles; diag_j is the
                  # diagonal-mask index or None. Fully-masked tiles skipped.
                  lst = [(tt, None) for tt in range(4)]  # encoder
                  if sc == 0:
                      lst += [(4 + j, j) for j in range(4)]
                  else:
                      lst += [(tt, None) for tt in range(4, 8)]
                      lst += [(8 + j, j) for j in range(4)]
                  return lst

              def emit_A(sc, h, pbuf):
                  tl = tiles_for(sc)
                  for k0 in range(0, len(tl), 2):
                      pr = psS.tile([128, 1024], fp32, tag="psS",
                                    name=f"sc{sc}h{h}p{k0}")
                      for half in range(2):
                          tt, dj = tl[k0 + half]
                          nc.tensor.matmul(
                              pr[:, ds(half * 512, 512)],
                              kT[:, ds(h * T + tt * 128, 128)],
                              qT[:, ds(h * S + sc * 512, 512)],
                              start=True,
                              stop=True,
                          )
                      nc.scalar.activation(
                          pbuf[:, ds(k0 * 512, 1024)], pr[:, :], AF.Exp
                      )
                      # causal zeroing of the diagonal-band tiles, post-exp
                      for half in range(2):
                          tt, dj = tl[k0 + half]
                          if dj is not None:
                              nc.vector.tensor_mul(
                                  pbuf[:, ds((k0 + half) * 512, 512)],
                                  pbuf[:, ds((k0 + half) * 512, 512)],
                                  dmsk[:, ds(dj * 512, 512)],
                              )

              def emit_B(sc, h, pbuf):
                  tl = tiles_for(sc)
                  n = len(tl)
                  zp = psZ.tile([128, 512], fp32, tag="psZ", name=f"z{sc}{h}")
                  for k, (tt, _) in enumerate(tl):
                      nc.tensor.matmul(
                          zp[:, :],
                          onesb[:, :],
                          pbuf[:, ds(k * 512, 512)],
                          start=(k == 0),
                          stop=(k == n - 1),
                      )
                  zr = zpool.tile([128, 512], fp32, tag="zr", name=f"zr{sc}{h}")
                  nc.vector.reciprocal_approx_fast(zr[:, :], zp[:, :])
                  pv = psV.tile([128, 512], fp32, tag="psV", name=f"pv{sc}{h}")
                  for k, (tt, _) in enumerate(tl):
                      nc.tensor.matmul(
                          pv[:, :],
                          vsb[:, ds(tt * O + h * 128, 128)],
                          pbuf[:, ds(k * 512, 512)],
                          start=(k == 0),
                          stop=(k == n - 1),
                      )
                  nc.vector.tensor_mul(
                      oT[:, ds(h * S + sc * 512, 512)], pv[:, :], zr[:, :]
                  )
                  # stage this head's slice into the A2A input: token quarter
                  # q goes to dest-rank rows q*128 and (4+q)*128
                  for q in range(4):
                      src = oT[:, ds(h * S + sc * 512 + q * 128, 128)]
                      for dup in range(2):
                          nc.sync.dma_start(
                              out=a2a_in[sc][
                                  ds((dup * 4 + q) * 128, 128),
                                  ds(h * 128, 128),
                              ],
                              in_=src,
                          )

              def fire_a2a(sc):
                  if no_collective:
                      nc.gpsimd.dma_start(
                          out=a2a_out[sc][:, :], in_=a2a_in[sc][:, :]
                      )
                  else:
                      nc.gpsimd.collective_compute(
                          "AllToAll",
                          mybir.AluOpType.bypass,
                          replica_groups=A2A_GROUP,
                          ins=[a2a_in[sc][:, :].opt()],
                          outs=[a2a_out[sc][:, :].opt()],
                      )

              # software pipeline: 2-head lookahead on A emits
              pend = []
              b_count = 0
              for sc in range(2):
                  for h in range(H):
                      pbuf = ppool.tile(
                          [128, 12 * 512], bf16, tag="p", name=f"pb{sc}{h}"
                      )
                      emit_A(sc, h, pbuf)
                      pend.append((sc, h, pbuf))
                      if len(pend) == 3:
                          s0, h0, pb0 = pend.pop(0)
                          emit_B(s0, h0, pb0)
                          b_count += 1
                          if b_count == 8:
                              fire_a2a(0)
                              # sc0 recv tiles: gpsimd is parked right
                              # behind the A2A#1 completion wait, so these
                              # run the moment it lands without stalling
                              # any compute-feeding queue
                              for i in range(8):
                                  nc.gpsimd.dma_start(
                                      out=rt0[i][:, :],
                                      in_=a2a_out[0][ds(i * 128, 128), :],
                                  )
              for s0, h0, pb0 in pend:
                  emit_B(s0, h0, pb0)
                  b_count += 1
              fire_a2a(1)

            # ---------------- phase 3: wo (y = oT_full.T @ woT) ----------
            # pass 1: d-chunks 0-1 (slabs resident from prefetch); sc0 first
            # (independent of A2A#2, hides it), then sc1. pass 2: d-chunks
            # 2-3 with freshly streamed slabs, sc0+sc1 jointly.
            with tc.tile_pool(name="wo2", bufs=8) as wo2, \
                 tc.tile_pool(name="rt1p", bufs=1) as rt1p, \
                 tc.tile_pool(name="ypool", bufs=1) as ypool, \
                 tc.tile_pool(name="psW", bufs=8, space="PSUM") as psW:
                rt1 = [
                    rt1p.tile([128, 1024], bf16, tag=f"rt1_{i}",
                              name=f"rt1_{i}")
                    for i in range(8)
                ]
                # sc1 recv: in consumption order (kt walks ig with both
                # batches), split over gpsimd (parked right behind the
                # A2A#2 completion wait) and sync (idle after staging)
                for k, i in enumerate((0, 4, 1, 5, 2, 6, 3, 7)):
                    (nc.gpsimd if k % 2 == 0 else nc.sync).dma_start(
                        out=rt1[i][:, :], in_=a2a_out[1][ds(i * 128, 128), :]
                    )
                rts = [rt0, rt1]
                yt = {
                    (sc, b): ypool.tile(
                        [128, DHALF], bf16, tag=f"y{sc}{b}", name=f"y{sc}{b}"
                    )
                    for sc in range(2) for b in range(2)
                }

                def wo_block(sc, dcs, slabs):
                    # psum[t, d] accumulated over all 32 k-tiles
                    pw = {
                        (b, dc): psW.tile([128, 512], fp32, tag="psW",
                                          name=f"pw{sc}{b}{dc}")
                        for b in range(2) for dc in dcs
                    }
                    for kt in range(32):
                        ig, h = divmod(kt, 8)
                        for b in range(2):
                            stat = rts[sc][b * 4 + ig][:, ds(h * 128, 128)]
                            for dc in dcs:
                                nc.tensor.matmul(
                                    pw[(b, dc)][:, :],
                                    stat,
                                    slabs[kt][:, ds((dc % 2) * 512, 512)],
                                    start=(kt == 0),
                                    stop=(kt == 31),
                                )
                    for b in range(2):
                        for dc in dcs:
                            nc.scalar.copy(
                                yt[(sc, b)][:, ds(dc * 512, 512)],
                                pw[(b, dc)][:, :],
                            )

                # pass 1 (resident slabs): sc0 then sc1
                wo_block(0, (0, 1), slab1)
                wo_block(1, (0, 1), slab1)
                # pass 2: stream the other d-half of each slab; one JOINT
                # kt loop over both slices so the rotating slab slots are
                # fully consumed before their reuse (no FIFO inversion)
                pw2 = {
                    (sc, b, dc): psW.tile([128, 512], fp32, tag="psW",
                                          name=f"p2w{sc}{b}{dc}")
                    for sc in range(2) for b in range(2) for dc in (2, 3)
                }
                for kt in range(32):
                    sl = wo2.tile([128, 1024], bf16, tag="sl2",
                                  name=f"slab2_{kt}")
                    (nc.scalar if kt % 2 == 0 else nc.sync).dma_start(
                        out=sl[:, :],
                        in_=P["wo_r"][:, ds(32768 + kt * 1024, 1024)],
                    )
                    ig, h = divmod(kt, 8)
                    for sc in range(2):
                        for b in range(2):
                            stat = rts[sc][b * 4 + ig][:, ds(h * 128, 128)]
                            for dc in (2, 3):
                                nc.tensor.matmul(
                                    pw2[(sc, b, dc)][:, :],
                                    stat,
                                    sl[:, ds((dc % 2) * 512, 512)],
                                    start=(kt == 0),
                                    stop=(kt == 31),
                                )
                for sc in range(2):
                    for b in range(2):
                        for dc in (2, 3):
                            nc.scalar.copy(
                                yt[(sc, b)][:, ds(dc * 512, 512)],
                                pw2[(sc, b, dc)][:, :],
                            )
                        nc.sync.dma_start(
                            out=out[ds(sc * 256 + b * 128, 128), :],
                            in_=yt[(sc, b)][:, :],
                        )


def _prep_in_maps(x, freqs_cos, freqs_sin, mask, encoder_output, wq, wk, wv, wo):
    x = np.asarray(x, np.float32)
    encoder_output = np.asarray(encoder_output, np.float32)
    freqs_cos = np.asarray(freqs_cos, np.float32)
    freqs_sin = np.asarray(freqs_sin, np.float32)
    wq = np.asarray(wq, np.float32)
    wk = np.asarray(wk, np.float32)
    wv = np.asarray(wv, np.float32)
    wo = np.asarray(wo, np.float32)

    def perm(w):  # deinterleave rope pairs per head: even dims first
        w4 = w.reshape(H, 64, 2, D)
        return np.ascontiguousarray(w4.transpose(0, 2, 1, 3)).reshape(O, D)

    def slab256(wT):  # [D, O] -> [128, 4*32*256]: pass p, dt n, col c
        w4 = wT.reshape(NDT, 128, 4, 256)            # [n, part, p, c]
        return np.ascontiguousarray(
            w4.transpose(1, 2, 0, 3)
        ).reshape(128, NDT * O)

    def slab512(wT):  # [D, O] -> [128, 2*32*512]: oc, dt n, col c
        w4 = wT.reshape(NDT, 128, 2, 512)
        return np.ascontiguousarray(
            w4.transpose(1, 2, 0, 3)
        ).reshape(128, NDT * O)

    alpha = 1.0 / np.sqrt(DH)
    cosT = freqs_cos.T  # [64, S]
    sinT = freqs_sin.T
    csq_cos = (np.concatenate([cosT, cosT], 0) * alpha).astype(BF16)
    csq_sin = (np.concatenate([-sinT, sinT], 0) * alpha).astype(BF16)
    csk_cos = np.concatenate([cosT, cosT], 0).astype(BF16)
    csk_sin = np.concatenate([-sinT, sinT], 0).astype(BF16)

    # 4 diagonal-band keep-masks (0/1, applied post-exp):
    # dmask[t, j*512+s] = 0 if s < t + j*128 else 1
    t_i = np.arange(128)[:, None]
    s_i = np.arange(512)[None, :]
    dmask = np.concatenate(
        [np.where(s_i < t_i + j * 128, 0.0, 1.0) for j in range(4)], axis=1
    ).astype(BF16)
    ones = np.ones((128, 128), BF16)

    # woT slabs: full wo.T (k = head*128+dh on partitions per k-tile), this
    # core's d-half, split into two 1024-wide passes
    woT = np.ascontiguousarray(wo.T).reshape(32, 128, D)  # [kt, dh, dout]

    in_maps = []
    for c in range(8):
        g, r = divmod(c, 4)
        dhalf = g
        sl = slice(r * O, (r + 1) * O)
        xhat = np.concatenate([encoder_output[g], x[g]], axis=0)  # [T, D]
        xhatT = xhat.T.astype(BF16)                               # [D, T]
        x_r = np.ascontiguousarray(
            xhatT.reshape(NDT, 128, T).transpose(1, 0, 2)
        ).reshape(128, NDT * T)
        wqT = perm(wq[sl]).T.astype(BF16)   # [D, O]
        wkT = perm(wk[sl]).T.astype(BF16)
        wvT = wv[sl].T.astype(BF16)
        wo_c = woT[:, :, dhalf * DHALF:(dhalf + 1) * DHALF]  # [32,128,2048]
        wo_r = np.ascontiguousarray(
            wo_c.reshape(32, 128, 2, 1024).transpose(1, 2, 0, 3)
        ).reshape(128, 2 * 32 * 1024).astype(BF16)
        in_maps.append(
            {
                "x_r": x_r,
                "wq_r": slab256(wqT),
                "wk_r": slab256(wkT),
                "wv_r": slab512(wvT),
                "wo_r": wo_r,
                "csq_cos": csq_cos,
                "csq_sin": csq_sin,
                "csk_cos": csk_cos,
                "csk_sin": csk_sin,
                "dmask": dmask,
                "ones": ones,
            }
        )
    return in_maps


def _gather(outs):
    full = np.zeros((2, S, D), np.float32)
    for c in range(8):
        g, q = divmod(c, 4)
        dhalf = g
        o = np.asarray(outs[c]).astype(np.float32)  # [512, 2048]
        for sc in range(2):
            for b in range(2):
                rows = o[sc * 256 + b * 128: sc * 256 + b * 128 + 128]
                full[b, sc * 512 + q * 128: sc * 512 + q * 128 + 128,
                     dhalf * DHALF:(dhalf + 1) * DHALF] = rows
    return full


def kernel(x, start_pos, freqs_cos, freqs_sin, mask, encoder_output, wq, wk, wv, wo):
    global LAST_EXEC_NS
    from concourse.bass_utils import run_bass_kernel_spmd

    if "nc" not in _CACHE:
        _CACHE["nc"] = _build()
    nc = _CACHE["nc"]

    in_maps = _prep_in_maps(
        x, freqs_cos, freqs_sin, mask, encoder_output, wq, wk, wv, wo
    )
    res = run_bass_kernel_spmd(nc, in_maps, core_ids=list(range(8)))
    LAST_EXEC_NS = res.exec_time_ns
    return _gather([res.results[c]["out"] for c in range(8)])
